# revision 38
# baseline (speedup 1.0000x reference)
"""AttentionBlock Trainium2 kernel.

Computes, per batch element b (data-parallel: one batch element per NeuronCore):
    xr = x[b] viewed as [C, S] (channels x tokens, S = H*W)
    QT = wq^T @ xr + bq   -> [D, S]  (d on partitions; heads = rows 64h..64h+63)
    KT = wk^T @ xr + bk   -> [D, S]
    V  = xr^T @ wv + bv   -> [S, D]  (tokens on partitions)
    per head h: ET[j, i] = (KT_h^T row-chunks) . QT_h   (keys on partitions)
                E = exp(ET / sqrt(C))        (no max-subtract; |args| < ~3)
                O'T[d, i] = sum_j V[j, d] E[j, i] ; Z[i] = sum_j E[j, i]
                (Z computed by a ones-column folded into the V stationary operand)
                OT[d, i] = O'T[d, i] / Z[i]
    y = wo^T @ OT + bo + xr   -> [C, S]

Heads are processed in pairs (2t, 2t+1): the pair's K/Q rows live in SBUF
partitions 0..63 / 64..127 of one d-chunk, so the two K=64 energy matmuls run
concurrently in disjoint PE row groups. Attention is pipelined per
(query-half, key-chunk): energy pair -> one Exp (N=1024, both heads) -> two
interleaved AV accumulations, with the V' stationary operand carrying a ones
column that lands Z = sum_j E[j, i] in psum row 0 for the softmax denominator.
Normalization of one query-half overlaps accumulation of the next; the final
projection's first 3 d-chunks overlap the last normalization.

Matmuls run as float32r by default (measured ~2x faster than fp32 on HW, rel
err ~2e-4); ATTN_MMDT=f32 selects exact-fp32 matmuls.
"""

import math
import os

import numpy as np

B = 8
C = 512
S = 1024  # 32*32 tokens
NH = 8
HD = 64
P = 128
CC = C // P  # 4 contraction chunks of 128
NI = 2  # S split into 2 chunks of 512 for matmul free dim
SC = S // P  # 8 token chunks of 128

# matmul operand dtype: f16 (default, 2x PE stream rate), f32r, or f32
MM_MODE = os.environ.get("ATTN_MMDT", "f32r")


def _emit(nc, tc, mybir, aps):
    import contextlib

    SKIP = set(os.environ.get("ATTN_SKIP", "").split(","))
    dump = os.environ.get("ATTN_DUMP", "")

    F32 = mybir.dt.float32
    F32R = mybir.dt.float32r
    MULT = mybir.AluOpType.mult
    EXP = mybir.ActivationFunctionType.Exp
    softmax_scale = 1.0 / math.sqrt(C)

    MMDT = {"f16": mybir.dt.float16, "f32r": F32R, "f32": F32}[MM_MODE]
    if MM_MODE == "f16":
        BITDT = mybir.dt.uint16
        ONE_BITS = 0x3C00  # 1.0 fp16
    else:
        BITDT = mybir.dt.uint32
        ONE_BITS = 0x3F800000  # 1.0f

    xb, wq, bq, wk, bk, wv, bv, wo, bo, y = (
        aps[k] for k in ("xb", "wq", "bq", "wk", "bk", "wv", "bv", "wo", "bo", "y")
    )
    xb_r = xb.rearrange("(cc p) s -> p cc s", p=P)
    y_r = y.rearrange("(cc p) s -> p cc s", p=P)
    wq_r = wq.rearrange("(cc p) d -> p cc d", p=P)
    wk_r = wk.rearrange("(cc p) d -> p cc d", p=P)
    wv_r = wv.rearrange("(cc p) d -> p cc d", p=P)
    wo_r = wo.rearrange("(dc p) c -> p dc c", p=P)
    bq_r = bq.rearrange("(dc p) -> p dc", p=P)
    bk_r = bk.rearrange("(dc p) -> p dc", p=P)
    bo_r = bo.rearrange("(cc p) -> p cc", p=P)

    with contextlib.ExitStack() as ctx:
        singles = ctx.enter_context(tc.tile_pool(name="singles", bufs=1))
        qkpool = ctx.enter_context(tc.tile_pool(name="qk", bufs=2))
        etpool = ctx.enter_context(tc.tile_pool(name="et", bufs=8))
        rbpool = ctx.enter_context(tc.tile_pool(name="rb", bufs=4))
        tmppool = ctx.enter_context(tc.tile_pool(name="tmp", bufs=3))
        pse = ctx.enter_context(tc.tile_pool(name="pse", bufs=2, space="PSUM"))
        psav = ctx.enter_context(tc.tile_pool(name="psav", bufs=2, space="PSUM"))

        # ---- load inputs to SBUF (per-chunk DMAs so compute starts early) ----
        cast_in = MM_MODE == "f16"  # dram side is f32/f32r; fp16 needs a cast

        def load(dma, out, in_):
            if cast_in:
                nc.gpsimd.dma_start(out=out, in_=in_)
            else:
                dma(out=out, in_=in_)

        xb_sb = singles.tile([P, CC, S], MMDT)
        for i in range(NI):
            for cc in range(CC):
                load(
                    nc.sync.dma_start,
                    xb_sb[:, cc, i * 512 : (i + 1) * 512],
                    xb_r[:, cc, i * 512 : (i + 1) * 512],
                )
        wq_sb = singles.tile([P, CC, C], MMDT)
        wk_sb = singles.tile([P, CC, C], MMDT)
        for cc in range(CC):
            load(nc.scalar.dma_start, wq_sb[:, cc], wq_r[:, cc])
            load(nc.scalar.dma_start, wk_sb[:, cc], wk_r[:, cc])
        wv_sb = singles.tile([P, CC, C], MMDT)
        for cc in range(CC):
            load(nc.scalar.dma_start, wv_sb[:, cc], wv_r[:, cc])
        wo_sb = singles.tile([P, CC, C], MMDT)
        load(nc.scalar.dma_start, wo_sb, wo_r)
        if cast_in:
            xb_res = singles.tile([P, CC, S], F32)
            for cc in range(CC):
                nc.sync.dma_start(out=xb_res[:, cc], in_=xb_r[:, cc].bitcast(F32))
        else:
            xb_res = xb_sb
        bq_sb = singles.tile([P, CC], F32)
        nc.sync.dma_start(out=bq_sb, in_=bq_r)
        bk_sb = singles.tile([P, CC], F32)
        nc.sync.dma_start(out=bk_sb, in_=bk_r)
        bo_sb = singles.tile([P, CC], F32)
        nc.sync.dma_start(out=bo_sb, in_=bo_r)
        bv_sb = singles.tile([1, C], MMDT)
        load(nc.sync.dma_start, bv_sb, bv[None, :])
        ones_row = singles.tile([1, P], MMDT)
        nc.vector.memset(ones_row.bitcast(BITDT), ONE_BITS)

        # V' layout: 128 columns per (jc, h): [ones | pad 63 | V (64)]. The AV
        # matmul (M=128) puts Z = sum_j E[j, i] at psum row 0 (ones column,
        # directly usable by the partition-0-only custom reciprocal), zeros at
        # rows 1..63, and O' at rows 64..127 for every head.
        Vp = singles.tile([P, SC, NH, P], MMDT)

        OTs = [singles.tile([P, S], MMDT, tag=f"ot{t}", name=f"ot{t}") for t in range(CC)]

        # PE warm-up: ~16 matmuls on zeros while input DMAs land, so the HAM
        # clock gate is already at 8/8 when real matmuls start
        warm = singles.tile([P, 512], MMDT)
        nc.vector.memset(warm.bitcast(BITDT), 0)
        ps_w = pse.tile([P, 2, 512], F32, tag="e")
        for _ in range(24):
            nc.tensor.matmul(ps_w[:, 0, 0:128], warm[:, 0:128], warm[:, 0:128])
        # zero-fill keeps the never-consumed pad columns finite (and CoreSim happy)
        nc.vector.memset(Vp.bitcast(BITDT), 0)
        nc.vector.memset(Vp[:, :, :, 0].bitcast(BITDT), ONE_BITS)

        def emit_v_projection_chunk(sc):
            # V[s, d] = xr^T @ wv + bv for one token chunk
            ps_v = pse.tile([P, 2, 512], F32, tag="e")
            for cc in range(CC):
                nc.tensor.matmul(
                    ps_v[:, 0],
                    xb_sb[:, cc, sc * P : (sc + 1) * P],
                    wv_sb[:, cc],
                    start=(cc == 0),
                    stop=False,
                )
            nc.tensor.matmul(ps_v[:, 0], ones_row, bv_sb, start=False, stop=True)
            psv_r = ps_v[:, 0].rearrange("p (h d) -> p h d", h=NH)
            nc.vector.tensor_copy(out=Vp[:, sc, :, 64:128], in_=psv_r)

        pending_norm = [None]
        pending_av = []

        def flush_av(depth=0):
            while len(pending_av) > depth:
                pending_av.pop(0)()

        def flush_norm():
            if pending_norm[0] is not None:
                pending_norm[0]()
                pending_norm[0] = None

        # ---- per head-pair t ----
        for t in range(CC):
            # QT/KT for heads (2t, 2t+1): per-i psum tiles, q/k halves
            qt = qkpool.tile([P, S], MMDT, tag="qt")
            kt = qkpool.tile([P, S], MMDT, tag="kt")
            for i in range(NI):
                sl = slice(i * 512, (i + 1) * 512)
                ps_p = pse.tile([P, 2, 512], F32, tag="e")
                for cc in range(CC):
                    xsl = xb_sb[:, cc, sl]
                    nc.tensor.matmul(
                        ps_p[:, 0],
                        wq_sb[:, cc, t * P : (t + 1) * P],
                        xsl,
                        start=(cc == 0),
                        stop=(cc == CC - 1),
                    )
                    nc.tensor.matmul(
                        ps_p[:, 1],
                        wk_sb[:, cc, t * P : (t + 1) * P],
                        xsl,
                        start=(cc == 0),
                        stop=(cc == CC - 1),
                    )
                nc.vector.tensor_scalar_add(qt[:, sl], ps_p[:, 0], bq_sb[:, t : t + 1])
                nc.vector.tensor_scalar_add(kt[:, sl], ps_p[:, 1], bk_sb[:, t : t + 1])
            flush_norm()
            if dump in ("qt", "kt"):
                tmpd = tmppool.tile([P, S], F32, tag="tmp")
                nc.vector.tensor_copy(out=tmpd, in_=qt if dump == "qt" else kt)
                nc.sync.dma_start(out=y_r[:, t], in_=tmpd)

            # energy -> exp -> AV, pipelined per (query-half i, key-chunk jc).
            # One [P, 2, 512] AV accumulator per query-half (double-buffered),
            # so the i=0 normalization overlaps the i=1 accumulation.
            h0, h1 = 2 * t, 2 * t + 1
            for i in range(NI):
                sl = slice(i * 512, (i + 1) * 512)
                ps_av = psav.tile([P, 2, 512], F32, tag="av")  # h0, h1
                for jc in range(SC):
                    if t == 0 and i == 0:
                        emit_v_projection_chunk(jc)
                    k0 = kt[0:64, jc * P : (jc + 1) * P]
                    k1 = kt[64:128, jc * P : (jc + 1) * P]
                    first, last = jc == 0, jc == SC - 1
                    ps_e = pse.tile([P, 2, 512], F32, tag="e")  # head-major
                    if "energy" not in SKIP:
                        nc.tensor.matmul(ps_e[:, 0], k0, qt[0:64, sl])
                        nc.tensor.matmul(ps_e[:, 1], k1, qt[64:128, sl])
                    et = etpool.tile([P, 2, 512], MMDT, tag="et")
                    if "exp" not in SKIP:
                        nc.scalar.activation(
                            out=et, in_=ps_e, func=EXP, scale=softmax_scale
                        )
                    # AV is emitted two units late so the in-order PE stream
                    # never waits on this unit's exp (keeps PE dense -> HAM
                    # stays at full clock); the skew also spans the i boundary
                    flush_av(depth=1)
                    if "av" not in SKIP:

                        def av(ps_av=ps_av, jc=jc, et=et, h0=h0, h1=h1,
                               first=first, last=last):
                            nc.tensor.matmul(
                                ps_av[:, 0], Vp[:, jc, h0], et[:, 0],
                                start=first, stop=last,
                            )
                            nc.tensor.matmul(
                                ps_av[:, 1], Vp[:, jc, h1], et[:, 1],
                                start=first, stop=last,
                            )

                        pending_av.append(av)

                if i == NI - 1:
                    flush_av()
                if dump == "av" and t == 0:
                    tmpd = tmppool.tile([P, S], F32, tag="tmp")
                    nc.vector.tensor_copy(
                        out=tmpd.rearrange("p (a s) -> p a s", a=2), in_=ps_av
                    )
                    nc.sync.dma_start(out=y_r[:, i], in_=tmpd)
                if "norm" not in SKIP:

                    def norm(t=t, i=i, sl=sl, ps_av=ps_av):
                        # Z sits at psum row 0 (custom recip is partition-0
                        # only); O' at rows 64..127 shifts down to the head's
                        # OT rows via the DVE multiply (builtin ops may cross
                        # partition bases)
                        rb = rbpool.tile([P, 2, 512], F32, tag="rb")
                        nc.vector.reciprocal_approx_fast(
                            out=rb[0:1], in_=ps_av[0:1]
                        )
                        nc.gpsimd.partition_broadcast(rb, rb[0:1], channels=128)
                        nc.vector.tensor_tensor(
                            OTs[t][0:64, sl], ps_av[64:128, 0], rb[0:64, 0], MULT
                        )
                        nc.vector.tensor_tensor(
                            OTs[t][64:128, sl], ps_av[64:128, 1], rb[64:128, 1], MULT
                        )

                    if pending_norm[0] is not None:
                        # i=0's norm runs while i=1 accumulates
                        flush_norm()
                    pending_norm[0] = norm

        if dump == "av":
            flush_norm()
            return
        if dump == "ot":
            flush_norm()
            for cc in range(CC):
                tmp = tmppool.tile([P, S], F32, tag="tmp")
                nc.vector.tensor_copy(out=tmp, in_=OTs[cc])
                nc.sync.dma_start(out=y_r[:, cc], in_=tmp)
            return
        if dump in ("qt", "kt"):
            flush_norm()
            return

        # ---- final projection + bias + residual ----
        # cc 0/1's dc=0..2 matmuls only need OTs[0..2], so they run on the PE
        # while the deferred t=3 normalization finishes on DVE/GpSimd
        if "final" in SKIP:
            flush_norm()
            return

        # All four cc accumulators live at once (two from the idle "av"
        # slots), so the 32 matmuls run back-to-back; dc=3 waits only on the
        # deferred t=3 normalization, which overlaps dc=0..2.
        ADD = mybir.AluOpType.add
        ps_fs = [
            pse.tile([P, 2, 512], F32, tag="e", name="psf0"),
            pse.tile([P, 2, 512], F32, tag="e", name="psf1"),
            psav.tile([P, 2, 512], F32, tag="av", name="psf2"),
            psav.tile([P, 2, 512], F32, tag="av", name="psf3"),
        ]
        for dc in range(CC - 1):
            for cc in range(CC):
                wo_sl = wo_sb[:, dc, cc * P : (cc + 1) * P]
                for i in range(NI):
                    sl = slice(i * 512, (i + 1) * 512)
                    nc.tensor.matmul(
                        ps_fs[cc][:, i], wo_sl, OTs[dc][:, sl],
                        start=(dc == 0), stop=False,
                    )
        flush_norm()
        dc = CC - 1
        for cc in range(CC):
            wo_sl = wo_sb[:, dc, cc * P : (cc + 1) * P]
            for i in range(NI):
                sl = slice(i * 512, (i + 1) * 512)
                nc.tensor.matmul(
                    ps_fs[cc][:, i], wo_sl, OTs[dc][:, sl],
                    start=False, stop=True,
                )
            tmp = tmppool.tile([P, S], F32, tag="tmp")
            nc.vector.scalar_tensor_tensor(
                out=tmp.rearrange("p (i s) -> p i s", i=2),
                in0=ps_fs[cc],
                scalar=bo_sb[:, cc : cc + 1],
                in1=xb_res[:, cc].rearrange("p (i s) -> p i s", i=2),
                op0=ADD,
                op1=ADD,
            )
            nc.sync.dma_start(out=y_r[:, cc], in_=tmp)


_NC_CACHE = {}


def _build():
    key = MM_MODE
    if key in _NC_CACHE:
        return _NC_CACHE[key]
    import concourse.bacc as bacc
    import concourse.mybir as mybir
    import concourse.tile as tile

    F32 = mybir.dt.float32
    MMDT = mybir.dt.float32r if MM_MODE == "f32r" else F32
    nc = bacc.Bacc("TRN2", target_bir_lowering=False, debug=False)
    aps = {}
    aps["xb"] = nc.dram_tensor("xb", (C, S), MMDT, kind="ExternalInput").ap()
    for name in ("wq", "wk", "wv"):
        aps[name] = nc.dram_tensor(name, (C, C), MMDT, kind="ExternalInput").ap()
    aps["wo"] = nc.dram_tensor("wo", (C, C), MMDT, kind="ExternalInput").ap()
    for name in ("bq", "bk", "bo"):
        aps[name] = nc.dram_tensor(name, (C,), F32, kind="ExternalInput").ap()
    aps["bv"] = nc.dram_tensor("bv", (C,), MMDT, kind="ExternalInput").ap()
    aps["y"] = nc.dram_tensor("y", (C, S), F32, kind="ExternalOutput").ap()
    with tile.TileContext(nc) as tc:
        _emit(nc, tc, mybir, aps)
    nc.compile()
    _NC_CACHE[key] = nc
    return nc


def kernel(x, wq, bq, wk, bk, wv, bv, wo, bo):
    from concourse import bass_utils

    nc = _build()
    x = np.ascontiguousarray(np.asarray(x, dtype=np.float32))
    xs = x.reshape(B, C, S)
    weights = {
        "wq": np.ascontiguousarray(np.asarray(wq, dtype=np.float32)),
        "bq": np.ascontiguousarray(np.asarray(bq, dtype=np.float32)),
        "wk": np.ascontiguousarray(np.asarray(wk, dtype=np.float32)),
        "bk": np.ascontiguousarray(np.asarray(bk, dtype=np.float32)),
        "wv": np.ascontiguousarray(np.asarray(wv, dtype=np.float32)),
        "bv": np.ascontiguousarray(np.asarray(bv, dtype=np.float32)),
        "wo": np.ascontiguousarray(np.asarray(wo, dtype=np.float32)),
        "bo": np.ascontiguousarray(np.asarray(bo, dtype=np.float32)),
    }
    in_maps = [{"xb": np.ascontiguousarray(xs[b]), **weights} for b in range(B)]
    res = bass_utils.run_bass_kernel_spmd(nc, in_maps, core_ids=list(range(B)))
    out = np.stack([r["y"] for r in res.results])
    return out.reshape(B, C, 32, 32)


# revision 39
# speedup vs baseline: 1.0797x; 1.0797x over previous
"""AttentionBlock Trainium2 kernel.

Computes, per batch element b (data-parallel: one batch element per NeuronCore):
    xr = x[b] viewed as [C, S] (channels x tokens, S = H*W)
    QT = wq^T @ xr + bq   -> [D, S]  (d on partitions; heads = rows 64h..64h+63)
    KT = wk^T @ xr + bk   -> [D, S]
    V  = xr^T @ wv + bv   -> [S, D]  (tokens on partitions)
    per head h: ET[j, i] = (KT_h^T row-chunks) . QT_h   (keys on partitions)
                E = exp(ET / sqrt(C))        (no max-subtract; |args| < ~3)
                O'T[d, i] = sum_j V[j, d] E[j, i] ; Z[i] = sum_j E[j, i]
                (Z computed by a ones-column folded into the V stationary operand)
                OT[d, i] = O'T[d, i] / Z[i]
    y = wo^T @ OT + bo + xr   -> [C, S]

Heads are processed in pairs (2t, 2t+1): the pair's K/Q rows live in SBUF
partitions 0..63 / 64..127 of one d-chunk, so the two K=64 energy matmuls run
concurrently in disjoint PE row groups. Attention is pipelined per
(query-half, key-chunk): energy pair -> one Exp (N=1024, both heads) -> two
interleaved AV accumulations, with the V' stationary operand carrying a ones
column that lands Z = sum_j E[j, i] in psum row 0 for the softmax denominator.
Normalization of one query-half overlaps accumulation of the next; the final
projection's first 3 d-chunks overlap the last normalization.

Matmuls run as float32r by default (measured ~2x faster than fp32 on HW, rel
err ~2e-4); ATTN_MMDT=f32 selects exact-fp32 matmuls.
"""

import math
import os

import numpy as np

B = 8
C = 512
S = 1024  # 32*32 tokens
NH = 8
HD = 64
P = 128
CC = C // P  # 4 contraction chunks of 128
NI = 2  # S split into 2 chunks of 512 for matmul free dim
SC = S // P  # 8 token chunks of 128

# matmul operand dtype: f16 (default, 2x PE stream rate), f32r, or f32
MM_MODE = os.environ.get("ATTN_MMDT", "f32r")


def _emit(nc, tc, mybir, aps):
    import contextlib

    SKIP = set(os.environ.get("ATTN_SKIP", "").split(","))
    dump = os.environ.get("ATTN_DUMP", "")

    F32 = mybir.dt.float32
    F32R = mybir.dt.float32r
    MULT = mybir.AluOpType.mult
    EXP = mybir.ActivationFunctionType.Exp
    softmax_scale = 1.0 / math.sqrt(C)

    MMDT = {"f16": mybir.dt.float16, "f32r": F32R, "f32": F32}[MM_MODE]
    if MM_MODE == "f16":
        BITDT = mybir.dt.uint16
        ONE_BITS = 0x3C00  # 1.0 fp16
    else:
        BITDT = mybir.dt.uint32
        ONE_BITS = 0x3F800000  # 1.0f

    xb, wq, bq, wk, bk, wv, bv, wo, bo, y = (
        aps[k] for k in ("xb", "wq", "bq", "wk", "bk", "wv", "bv", "wo", "bo", "y")
    )
    xb_r = xb.rearrange("(cc p) s -> p cc s", p=P)
    y_r = y.rearrange("(cc p) s -> p cc s", p=P)
    wq_r = wq.rearrange("(cc p) d -> p cc d", p=P)
    wk_r = wk.rearrange("(cc p) d -> p cc d", p=P)
    wv_r = wv.rearrange("(cc p) d -> p cc d", p=P)
    wo_r = wo.rearrange("(dc p) c -> p dc c", p=P)
    bq_r = bq.rearrange("(dc p) -> p dc", p=P)
    bk_r = bk.rearrange("(dc p) -> p dc", p=P)
    bo_r = bo.rearrange("(cc p) -> p cc", p=P)

    with contextlib.ExitStack() as ctx:
        singles = ctx.enter_context(tc.tile_pool(name="singles", bufs=1))
        qkpool = ctx.enter_context(tc.tile_pool(name="qk", bufs=2))
        etpool = ctx.enter_context(tc.tile_pool(name="et", bufs=8))
        rbpool = ctx.enter_context(tc.tile_pool(name="rb", bufs=4))
        tmppool = ctx.enter_context(tc.tile_pool(name="tmp", bufs=3))
        pse = ctx.enter_context(tc.tile_pool(name="pse", bufs=2, space="PSUM"))
        psav = ctx.enter_context(tc.tile_pool(name="psav", bufs=2, space="PSUM"))

        # ---- load inputs to SBUF (per-chunk DMAs so compute starts early) ----
        cast_in = MM_MODE == "f16"  # dram side is f32/f32r; fp16 needs a cast

        def load(dma, out, in_):
            if cast_in:
                nc.gpsimd.dma_start(out=out, in_=in_)
            else:
                dma(out=out, in_=in_)

        xb_sb = singles.tile([P, CC, S], MMDT)
        for i in range(NI):
            for cc in range(CC):
                load(
                    nc.sync.dma_start,
                    xb_sb[:, cc, i * 512 : (i + 1) * 512],
                    xb_r[:, cc, i * 512 : (i + 1) * 512],
                )
        wq_sb = singles.tile([P, CC, C], MMDT)
        wk_sb = singles.tile([P, CC, C], MMDT)
        for cc in range(CC):
            load(nc.scalar.dma_start, wq_sb[:, cc], wq_r[:, cc])
            load(nc.scalar.dma_start, wk_sb[:, cc], wk_r[:, cc])
        wv_sb = singles.tile([P, CC, C], MMDT)
        for cc in range(CC):
            load(nc.scalar.dma_start, wv_sb[:, cc], wv_r[:, cc])
        wo_sb = singles.tile([P, CC, C], MMDT)
        load(nc.scalar.dma_start, wo_sb, wo_r)
        if cast_in:
            xb_res = singles.tile([P, CC, S], F32)
            for cc in range(CC):
                nc.sync.dma_start(out=xb_res[:, cc], in_=xb_r[:, cc].bitcast(F32))
        else:
            xb_res = xb_sb
        bq_sb = singles.tile([P, CC], F32)
        nc.sync.dma_start(out=bq_sb, in_=bq_r)
        bk_sb = singles.tile([P, CC], F32)
        nc.sync.dma_start(out=bk_sb, in_=bk_r)
        bo_sb = singles.tile([P, CC], F32)
        nc.sync.dma_start(out=bo_sb, in_=bo_r)
        bv_sb = singles.tile([1, C], MMDT)
        load(nc.sync.dma_start, bv_sb, bv[None, :])
        ones_row = singles.tile([1, P], MMDT)
        nc.vector.memset(ones_row.bitcast(BITDT), ONE_BITS)

        # V' layout: 128 columns per (jc, h): [ones | pad 63 | V (64)]. The AV
        # matmul (M=128) puts Z = sum_j E[j, i] at psum row 0 (ones column,
        # directly usable by the partition-0-only custom reciprocal), zeros at
        # rows 1..63, and O' at rows 64..127 for every head.
        Vp = singles.tile([P, SC, NH, P], MMDT)

        OTs = [singles.tile([P, S], MMDT, tag=f"ot{t}", name=f"ot{t}") for t in range(CC)]

        # PE warm-up: ~16 matmuls on zeros while input DMAs land, so the HAM
        # clock gate is already at 8/8 when real matmuls start
        warm = singles.tile([P, 512], MMDT)
        nc.vector.memset(warm.bitcast(BITDT), 0)
        ps_w = pse.tile([P, 2, 512], F32, tag="e")
        for _ in range(24):
            nc.tensor.matmul(ps_w[:, 0, 0:128], warm[:, 0:128], warm[:, 0:128])
        # zero-fill keeps the never-consumed pad columns finite (and CoreSim happy)
        nc.vector.memset(Vp.bitcast(BITDT), 0)
        nc.vector.memset(Vp[:, :, :, 0].bitcast(BITDT), ONE_BITS)

        def emit_v_projection_chunk(sc):
            # V[s, d] = xr^T @ wv + bv for one token chunk
            ps_v = pse.tile([P, 2, 512], F32, tag="e")
            for cc in range(CC):
                nc.tensor.matmul(
                    ps_v[:, 0],
                    xb_sb[:, cc, sc * P : (sc + 1) * P],
                    wv_sb[:, cc],
                    start=(cc == 0),
                    stop=False,
                )
            nc.tensor.matmul(ps_v[:, 0], ones_row, bv_sb, start=False, stop=True)
            psv_r = ps_v[:, 0].rearrange("p (h d) -> p h d", h=NH)
            nc.vector.tensor_copy(out=Vp[:, sc, :, 64:128], in_=psv_r)

        pending_norm = [None]
        pending_av = []

        def flush_av(depth=0):
            while len(pending_av) > depth:
                pending_av.pop(0)()

        def flush_norm():
            if pending_norm[0] is not None:
                pending_norm[0]()
                pending_norm[0] = None

        # ---- per head-pair t ----
        for t in range(CC):
            # QT/KT for heads (2t, 2t+1): per-i psum tiles, q/k halves
            qt = qkpool.tile([P, S], MMDT, tag="qt")
            kt = qkpool.tile([P, S], MMDT, tag="kt")
            for i in range(NI):
                sl = slice(i * 512, (i + 1) * 512)
                ps_p = pse.tile([P, 2, 512], F32, tag="e")
                for cc in range(CC):
                    xsl = xb_sb[:, cc, sl]
                    nc.tensor.matmul(
                        ps_p[:, 0],
                        wq_sb[:, cc, t * P : (t + 1) * P],
                        xsl,
                        start=(cc == 0),
                        stop=(cc == CC - 1),
                    )
                    nc.tensor.matmul(
                        ps_p[:, 1],
                        wk_sb[:, cc, t * P : (t + 1) * P],
                        xsl,
                        start=(cc == 0),
                        stop=(cc == CC - 1),
                    )
                nc.vector.tensor_scalar_add(qt[:, sl], ps_p[:, 0], bq_sb[:, t : t + 1])
                nc.vector.tensor_scalar_add(kt[:, sl], ps_p[:, 1], bk_sb[:, t : t + 1])
            flush_norm()
            if dump in ("qt", "kt"):
                tmpd = tmppool.tile([P, S], F32, tag="tmp")
                nc.vector.tensor_copy(out=tmpd, in_=qt if dump == "qt" else kt)
                nc.sync.dma_start(out=y_r[:, t], in_=tmpd)

            # energy -> exp -> AV, pipelined per (query-half i, key-chunk jc).
            # One [P, 2, 512] AV accumulator per query-half (double-buffered),
            # so the i=0 normalization overlaps the i=1 accumulation.
            h0, h1 = 2 * t, 2 * t + 1
            for i in range(NI):
                sl = slice(i * 512, (i + 1) * 512)
                ps_av = psav.tile([P, 2, 512], F32, tag="av")  # h0, h1
                for jc in range(SC):
                    if t == 0 and i == 0:
                        emit_v_projection_chunk(jc)
                    k0 = kt[0:64, jc * P : (jc + 1) * P]
                    k1 = kt[64:128, jc * P : (jc + 1) * P]
                    first, last = jc == 0, jc == SC - 1
                    ps_e = pse.tile([P, 2, 512], F32, tag="e")  # head-major
                    if "energy" not in SKIP:
                        nc.tensor.matmul(ps_e[:, 0], k0, qt[0:64, sl])
                        nc.tensor.matmul(ps_e[:, 1], k1, qt[64:128, sl])
                    et = etpool.tile([P, 2, 512], MMDT, tag="et")
                    if "exp" not in SKIP:
                        nc.scalar.activation(
                            out=et, in_=ps_e, func=EXP, scale=softmax_scale
                        )
                    # AV is emitted one unit late so the in-order PE stream
                    # never waits on this unit's exp (keeps PE dense -> HAM
                    # stays at full clock)
                    flush_av(depth=0)
                    if "av" not in SKIP:

                        def av(ps_av=ps_av, jc=jc, et=et, h0=h0, h1=h1,
                               first=first, last=last):
                            nc.tensor.matmul(
                                ps_av[:, 0], Vp[:, jc, h0], et[:, 0],
                                start=first, stop=last,
                            )
                            nc.tensor.matmul(
                                ps_av[:, 1], Vp[:, jc, h1], et[:, 1],
                                start=first, stop=last,
                            )

                        pending_av.append(av)

                flush_av()
                if dump == "av" and t == 0:
                    tmpd = tmppool.tile([P, S], F32, tag="tmp")
                    nc.vector.tensor_copy(
                        out=tmpd.rearrange("p (a s) -> p a s", a=2), in_=ps_av
                    )
                    nc.sync.dma_start(out=y_r[:, i], in_=tmpd)
                if "norm" not in SKIP:

                    def norm(t=t, i=i, sl=sl, ps_av=ps_av):
                        # Z sits at psum row 0 (custom recip is partition-0
                        # only); O' at rows 64..127 shifts down to the head's
                        # OT rows via the DVE multiply (builtin ops may cross
                        # partition bases)
                        rb = rbpool.tile([P, 2, 512], F32, tag="rb")
                        nc.vector.reciprocal_approx_fast(
                            out=rb[0:1], in_=ps_av[0:1]
                        )
                        nc.gpsimd.partition_broadcast(rb, rb[0:1], channels=128)
                        nc.vector.tensor_tensor(
                            OTs[t][0:64, sl], ps_av[64:128, 0], rb[0:64, 0], MULT
                        )
                        nc.vector.tensor_tensor(
                            OTs[t][64:128, sl], ps_av[64:128, 1], rb[64:128, 1], MULT
                        )

                    if pending_norm[0] is not None:
                        # i=0's norm runs while i=1 accumulates
                        flush_norm()
                    pending_norm[0] = norm

        if dump == "av":
            flush_norm()
            return
        if dump == "ot":
            flush_norm()
            for cc in range(CC):
                tmp = tmppool.tile([P, S], F32, tag="tmp")
                nc.vector.tensor_copy(out=tmp, in_=OTs[cc])
                nc.sync.dma_start(out=y_r[:, cc], in_=tmp)
            return
        if dump in ("qt", "kt"):
            flush_norm()
            return

        # ---- final projection + bias + residual ----
        # cc 0/1's dc=0..2 matmuls only need OTs[0..2], so they run on the PE
        # while the deferred t=3 normalization finishes on DVE/GpSimd
        if "final" in SKIP:
            flush_norm()
            return

        # All four cc accumulators live at once (two from the idle "av"
        # slots), so the 32 matmuls run back-to-back; dc=3 waits only on the
        # deferred t=3 normalization, which overlaps dc=0..2.
        ADD = mybir.AluOpType.add
        ps_fs = [
            pse.tile([P, 2, 512], F32, tag="e", name="psf0"),
            pse.tile([P, 2, 512], F32, tag="e", name="psf1"),
            psav.tile([P, 2, 512], F32, tag="av", name="psf2"),
            psav.tile([P, 2, 512], F32, tag="av", name="psf3"),
        ]
        for dc in range(CC - 1):
            for cc in range(CC):
                wo_sl = wo_sb[:, dc, cc * P : (cc + 1) * P]
                for i in range(NI):
                    sl = slice(i * 512, (i + 1) * 512)
                    nc.tensor.matmul(
                        ps_fs[cc][:, i], wo_sl, OTs[dc][:, sl],
                        start=(dc == 0), stop=False,
                    )
        flush_norm()
        dc = CC - 1
        for cc in range(CC):
            wo_sl = wo_sb[:, dc, cc * P : (cc + 1) * P]
            for i in range(NI):
                sl = slice(i * 512, (i + 1) * 512)
                nc.tensor.matmul(
                    ps_fs[cc][:, i], wo_sl, OTs[dc][:, sl],
                    start=False, stop=True,
                )
            tmp = tmppool.tile([P, S], F32, tag="tmp")
            nc.vector.scalar_tensor_tensor(
                out=tmp.rearrange("p (i s) -> p i s", i=2),
                in0=ps_fs[cc],
                scalar=bo_sb[:, cc : cc + 1],
                in1=xb_res[:, cc].rearrange("p (i s) -> p i s", i=2),
                op0=ADD,
                op1=ADD,
            )
            nc.sync.dma_start(out=y_r[:, cc], in_=tmp)


_NC_CACHE = {}


def _build():
    key = MM_MODE
    if key in _NC_CACHE:
        return _NC_CACHE[key]
    import concourse.bacc as bacc
    import concourse.mybir as mybir
    import concourse.tile as tile

    F32 = mybir.dt.float32
    MMDT = mybir.dt.float32r if MM_MODE == "f32r" else F32
    nc = bacc.Bacc("TRN2", target_bir_lowering=False, debug=False)
    aps = {}
    aps["xb"] = nc.dram_tensor("xb", (C, S), MMDT, kind="ExternalInput").ap()
    for name in ("wq", "wk", "wv"):
        aps[name] = nc.dram_tensor(name, (C, C), MMDT, kind="ExternalInput").ap()
    aps["wo"] = nc.dram_tensor("wo", (C, C), MMDT, kind="ExternalInput").ap()
    for name in ("bq", "bk", "bo"):
        aps[name] = nc.dram_tensor(name, (C,), F32, kind="ExternalInput").ap()
    aps["bv"] = nc.dram_tensor("bv", (C,), MMDT, kind="ExternalInput").ap()
    aps["y"] = nc.dram_tensor("y", (C, S), F32, kind="ExternalOutput").ap()
    with tile.TileContext(nc) as tc:
        _emit(nc, tc, mybir, aps)
    nc.compile()
    _NC_CACHE[key] = nc
    return nc


def kernel(x, wq, bq, wk, bk, wv, bv, wo, bo):
    from concourse import bass_utils

    nc = _build()
    x = np.ascontiguousarray(np.asarray(x, dtype=np.float32))
    xs = x.reshape(B, C, S)
    weights = {
        "wq": np.ascontiguousarray(np.asarray(wq, dtype=np.float32)),
        "bq": np.ascontiguousarray(np.asarray(bq, dtype=np.float32)),
        "wk": np.ascontiguousarray(np.asarray(wk, dtype=np.float32)),
        "bk": np.ascontiguousarray(np.asarray(bk, dtype=np.float32)),
        "wv": np.ascontiguousarray(np.asarray(wv, dtype=np.float32)),
        "bv": np.ascontiguousarray(np.asarray(bv, dtype=np.float32)),
        "wo": np.ascontiguousarray(np.asarray(wo, dtype=np.float32)),
        "bo": np.ascontiguousarray(np.asarray(bo, dtype=np.float32)),
    }
    in_maps = [{"xb": np.ascontiguousarray(xs[b]), **weights} for b in range(B)]
    res = bass_utils.run_bass_kernel_spmd(nc, in_maps, core_ids=list(range(B)))
    out = np.stack([r["y"] for r in res.results])
    return out.reshape(B, C, 32, 32)
